# revision 1
# baseline (speedup 1.0000x reference)
"""Multi-head attention (B=2, S=2048, D=1024, H=16) on 8 TRN2 cores.

Sharding: core c -> batch b = c//4, head-group g = c%4 (heads 4g..4g+3,
projection dims 256g..256g+256). Each core computes a partial output
projection over its own 256 head-dims, then per-512-token-chunk 4-core
ReduceScatter(add) sums the partials and hands each core output dims
256r..256r+256; the collectives overlap later compute.

Device pipeline per core:
  1. q^T,k^T projections d-major [128, 2048] head-pair tiles; v
     projection token-major [2048, 4*68] with 4 aug columns per head
     (col 64+h is ones for head h -> per-head softmax denominator row).
  2. Head-outer software pipeline: per (sk tile, s4 half) a 2-bank QK
     matmul pair -> one fused exp over [128,1024]; AV accumulation into
     4 live [68,512] PSUM chunks lags one half-step so PE never waits
     on ACT. Denominators land on pso rows 64..67; summed into den_sb
     rows 0..3 (aligned 64-partition DVE shift).
  3. One reciprocal_approx_fast over [4,2048], selector matmuls
     broadcast per-(h,s4) recip rows to 64 partitions, in-place
     normalize at_sb; per s4: out-proj matmuls + ReduceScatter(add).
"""

import numpy as np
from contextlib import ExitStack

import concourse.bass as bass
import concourse.tile as tile
from concourse import mybir
from concourse._compat import with_exitstack

F32 = mybir.dt.float32
R32 = mybir.dt.float32r
AF = mybir.ActivationFunctionType


B, S, D = 2, 2048, 1024
NCORES, GROUP = 8, 4
DG = D // GROUP          # 256 projection dims per core
NH = 4                   # heads per core
DH = 64
SQ = 512                 # sq chunk (PSUM bank width in fp32)
NSQ = S // SQ            # 4
SKT = 128                # sk tile
NSK = S // SKT           # 16
KT = 128                 # contraction tile
NKT = D // KT            # 8
NAUG = 4                 # aug ones-columns per head (col 64+h hot)
VW = DH + NAUG           # 68 v_aug cols per head
SCALE = 0.125            # 1/sqrt(64)


@with_exitstack
def _mha(ctx: ExitStack, tc: "tile.TileContext", out, xq, xk, xv, wq, wk, wv, wo,
         maskb, sel, aug):
    nc = tc.nc
    P = 128

    # ---- persistent SBUF ----
    persist = ctx.enter_context(tc.tile_pool(name="persist", bufs=1))

    def T(shape, name, dt=F32):
        return persist.tile(shape, dt, name=name, tag=name)

    wq_sb = T([P, NKT * DG], "wq_sb", R32)
    wk_sb = T([P, NKT * DG], "wk_sb", R32)
    wv_sb = T([P, NKT * DG], "wv_sb", R32)
    wo_sb = T([P, 2 * D], "wo_sb", R32)
    mask_sb = T([P, NSK], "mask_sb")
    q_sb = T([P, 2 * S], "q_sb", R32)
    k_sb = T([P, 2 * S], "k_sb", R32)
    v_sb = T([P, NSK * NH * VW], "v_sb", R32)
    at_sb = T([P, 2 * S], "at_sb", R32)
    den_sb = T([NAUG, S], "den_sb")
    rec_f = T([NAUG, S], "rec_f")
    rec_r = T([NAUG, S], "rec_r", R32)
    sel_sb = T([NAUG, NH * DH], "sel_sb", R32)
    aug_sb = T([P, NH * NAUG], "aug_sb")
    nc.vector.memset(den_sb[:], 0.0)

    for k in range(NKT):
        nc.sync.dma_start(wq_sb[:, bass.ts(k, DG)], wq[bass.ts(k, P), :])
        nc.sync.dma_start(wk_sb[:, bass.ts(k, DG)], wk[bass.ts(k, P), :])
        nc.sync.dma_start(wv_sb[:, bass.ts(k, DG)], wv[bass.ts(k, P), :])
    for k in range(2):
        nc.sync.dma_start(wo_sb[:, bass.ts(k, D)], wo[bass.ts(k, P), :])
    nc.sync.dma_start(mask_sb[:], maskb[:, :])
    nc.sync.dma_start(sel_sb[:], sel[:, :])
    nc.sync.dma_start(aug_sb[:], aug[:, :])

    # ---- phase 1: projections ----
    with tc.tile_pool(name="xin", bufs=3) as xin_pool, \
         tc.tile_pool(name="ppqk", bufs=4, space="PSUM") as ppqk, \
         tc.tile_pool(name="ppv", bufs=2, space="PSUM") as ppv:
        for xdram, wsb, dst in ((xq, wq_sb, q_sb), (xk, wk_sb, k_sb)):
            for s4 in range(NSQ):
                xin = xin_pool.tile([P, NKT * SQ], R32, name="xin")
                for k in range(NKT):
                    nc.sync.dma_start(
                        xin[:, bass.ts(k, SQ)],
                        xdram[bass.ts(k, P), bass.ts(s4, SQ)],
                    )
                for d2 in range(2):
                    ps = ppqk.tile([P, SQ], F32, name="ps")
                    for k in range(NKT):
                        nc.tensor.matmul(
                            ps[:],
                            lhsT=wsb[:, bass.ds(k * DG + d2 * P, P)],
                            rhs=xin[:, bass.ts(k, SQ)],
                            start=(k == 0),
                            stop=(k == NKT - 1),
                        )
                    nc.vector.tensor_copy(
                        dst[:, bass.ds(d2 * S + s4 * SQ, SQ)], ps[:]
                    )

        for st in range(NSK):
            vin = xin_pool.tile([P, NKT * SKT], R32, name="vin")
            for k in range(NKT):
                nc.sync.dma_start(
                    vin[:, bass.ts(k, SKT)],
                    xv[bass.ts(k, P), bass.ts(st, SKT)],
                )
            psv = ppv.tile([P, DG], F32, name="psv")
            for k in range(NKT):
                nc.tensor.matmul(
                    psv[:],
                    lhsT=vin[:, bass.ts(k, SKT)],
                    rhs=wv_sb[:, bass.ts(k, DG)],
                    start=(k == 0),
                    stop=(k == NKT - 1),
                )
            base = st * NH * VW
            for h in range(NH):
                nc.vector.tensor_copy(
                    v_sb[:, bass.ds(base + h * VW, DH)], psv[:, bass.ts(h, DH)]
                )
                nc.vector.tensor_copy(
                    v_sb[:, bass.ds(base + h * VW + DH, NAUG)],
                    aug_sb[:, bass.ts(h, NAUG)],
                )

    # ---- phase 2: attention (h-outer, lag-1 AV pipeline) ----
    with tc.tile_pool(name="expp", bufs=3) as exp_pool, \
         tc.tile_pool(name="pslp", bufs=2, space="PSUM") as psl_pool, \
         tc.tile_pool(name="psop", bufs=1, space="PSUM") as pso_pool:
        for h in range(NH):
            pr, po = h // 2, (h % 2) * DH
            pso = [pso_pool.tile([VW, SQ], F32, name=f"pso{i}") for i in range(NSQ)]

            def emit_av(item):
                ex_t, sk_i, half_i = item
                for i in range(2):
                    s4 = half_i * 2 + i
                    nc.tensor.matmul(
                        pso[s4][:],
                        lhsT=v_sb[:, bass.ds(sk_i * NH * VW + h * VW, VW)],
                        rhs=ex_t[:, bass.ts(i, SQ)],
                        start=(sk_i == 0),
                        stop=(sk_i == NSK - 1),
                        skip_group_check=True,
                    )

            prev = None
            for sk in range(NSK):
                for half in range(2):
                    psl = psl_pool.tile([P, 2 * SQ], F32, name="psl")
                    for i in range(2):
                        s4 = half * 2 + i
                        nc.tensor.matmul(
                            psl[:, bass.ts(i, SQ)],
                            lhsT=k_sb[bass.ds(po, DH), bass.ds(pr * S + sk * SKT, SKT)],
                            rhs=q_sb[bass.ds(po, DH), bass.ds(pr * S + s4 * SQ, SQ)],
                            start=True,
                            stop=True,
                        )
                    ex = exp_pool.tile([P, 2 * SQ], R32, name="ex")
                    nc.scalar.activation(
                        ex[:],
                        psl[:],
                        AF.Exp,
                        bias=mask_sb[:, bass.ds(sk, 1)],
                        scale=SCALE,
                    )
                    if prev is not None:
                        emit_av(prev)
                    prev = (ex, sk, half)
            emit_av(prev)

            for s4 in range(NSQ):
                nc.vector.tensor_copy(
                    at_sb[bass.ds(po, DH), bass.ds(pr * S + s4 * SQ, SQ)],
                    pso[s4][bass.ds(0, DH), :],
                )
                nc.vector.tensor_add(
                    den_sb[:, bass.ts(s4, SQ)],
                    den_sb[:, bass.ts(s4, SQ)],
                    pso[s4][bass.ds(DH, NAUG), :],
                )

    # ---- normalize + phase 3: out-proj with per-chunk ReduceScatter ----
    nc.vector.reciprocal_approx_fast(rec_f[:], den_sb[:])
    nc.vector.tensor_copy(rec_r[:], rec_f[:])

    dram = ctx.enter_context(tc.tile_pool(name="dram", bufs=1, space="DRAM"))
    rs_in = [dram.tile([D, SQ], F32, name=f"rs_in{i}", tag=f"rs_in{i}")
             for i in range(NSQ)]
    rs_out = [dram.tile([DG, SQ], F32, name=f"rs_out{i}", tag=f"rs_out{i}")
              for i in range(NSQ)]

    with tc.tile_pool(name="psb", bufs=4, space="PSUM") as psb_pool, \
         tc.tile_pool(name="fin", bufs=2) as fin_pool, \
         tc.tile_pool(name="psf", bufs=2, space="PSUM") as psf_pool:
        for s4 in range(NSQ):
            for h in range(NH):
                pr, po = h // 2, (h % 2) * DH
                pb = psb_pool.tile([DH, SQ], F32, name="pb")
                nc.tensor.matmul(
                    pb[:],
                    lhsT=sel_sb[:, bass.ts(h, DH)],
                    rhs=rec_r[:, bass.ds(s4 * SQ, SQ)],
                    start=True,
                    stop=True,
                )
                nc.vector.tensor_mul(
                    at_sb[bass.ds(po, DH), bass.ds(pr * S + s4 * SQ, SQ)],
                    at_sb[bass.ds(po, DH), bass.ds(pr * S + s4 * SQ, SQ)],
                    pb[:],
                )

        for s4 in range(NSQ):
            for do8 in range(NKT):
                psf = psf_pool.tile([P, SQ], F32, name="psf")
                for kt in range(2):
                    nc.tensor.matmul(
                        psf[:],
                        lhsT=wo_sb[:, bass.ds(kt * D + do8 * P, P)],
                        rhs=at_sb[:, bass.ds(kt * S + s4 * SQ, SQ)],
                        start=(kt == 0),
                        stop=(kt == 1),
                    )
                ot = fin_pool.tile([P, SQ], F32, name="ot")
                nc.scalar.activation(ot[:], psf[:], AF.Copy)
                nc.sync.dma_start(rs_in[s4][bass.ts(do8, P), :], ot[:])
            nc.gpsimd.collective_compute(
                "ReduceScatter",
                mybir.AluOpType.add,
                replica_groups=[[0, 1, 2, 3], [4, 5, 6, 7]],
                ins=[rs_in[s4].opt()],
                outs=[rs_out[s4].opt()],
            )
            nc.sync.dma_start(out[:, bass.ts(s4, SQ)], rs_out[s4][:])


def build_program():
    from concourse import bacc

    nc = bacc.Bacc("TRN2", target_bir_lowering=False, debug=False, num_devices=NCORES)
    aps = {}
    for nm, shp, dt in (
        ("xq", [D, S], R32),
        ("xk", [D, S], R32),
        ("xv", [D, S], R32),
        ("wq", [D, DG], R32),
        ("wk", [D, DG], R32),
        ("wv", [D, DG], R32),
        ("wo", [DG, D], R32),
        ("maskb", [128, NSK], F32),
        ("sel", [NAUG, NH * DH], R32),
        ("aug", [128, NH * NAUG], F32),
    ):
        aps[nm] = nc.dram_tensor(nm, shp, dt, kind="ExternalInput").ap()
    out = nc.dram_tensor("out", [DG, S], F32, kind="ExternalOutput").ap()
    with tile.TileContext(nc) as tc:
        _mha(tc, out, **aps)
    nc.finalize()
    return nc


_NC_CACHE = None


def _get_program():
    global _NC_CACHE
    if _NC_CACHE is None:
        _NC_CACHE = build_program()
    return _NC_CACHE


def make_in_maps(query, key, value, mask, Wq, Wk, Wv, Wo):
    xT = {}
    for b in range(B):
        xT[("q", b)] = np.ascontiguousarray(query[b].T, dtype=np.float32)
        xT[("k", b)] = np.ascontiguousarray(key[b].T, dtype=np.float32)
        xT[("v", b)] = np.ascontiguousarray(value[b].T, dtype=np.float32)
    sel = np.zeros((NAUG, NH * DH), dtype=np.float32)
    aug = np.zeros((128, NH * NAUG), dtype=np.float32)
    for h in range(NH):
        sel[h, h * DH:(h + 1) * DH] = 1.0
        aug[:, h * NAUG + h] = 1.0
    in_maps = []
    for c in range(NCORES):
        b, g = divmod(c, GROUP)
        mrow = (mask[b].astype(np.float32) * np.float32(-1e9)).astype(np.float32)
        in_maps.append(
            {
                "xq": xT[("q", b)],
                "xk": xT[("k", b)],
                "xv": xT[("v", b)],
                "wq": np.ascontiguousarray(Wq[g * DG:(g + 1) * DG, :].T, dtype=np.float32),
                "wk": np.ascontiguousarray(Wk[g * DG:(g + 1) * DG, :].T, dtype=np.float32),
                "wv": np.ascontiguousarray(Wv[g * DG:(g + 1) * DG, :].T, dtype=np.float32),
                "wo": np.ascontiguousarray(Wo[:, g * DG:(g + 1) * DG].T, dtype=np.float32),
                "maskb": np.ascontiguousarray(mrow.reshape(NSK, 128).T),
                "sel": sel,
                "aug": aug,
            }
        )
    return in_maps


def assemble_output(results):
    out = np.empty((B, S, D), dtype=np.float32)
    for c in range(NCORES):
        b, r = divmod(c, GROUP)
        out[b, :, r * DG:(r + 1) * DG] = results[c]["out"].T
    return out


def kernel(query, key, value, mask, Wq, bq, Wk, bk, Wv, bv, Wo, bo, trace=False):
    from concourse.bass_utils import run_bass_kernel_spmd

    nc = _get_program()
    in_maps = make_in_maps(
        np.asarray(query), np.asarray(key), np.asarray(value), np.asarray(mask),
        np.asarray(Wq), np.asarray(Wk), np.asarray(Wv), np.asarray(Wo),
    )
    br = run_bass_kernel_spmd(nc, in_maps, list(range(NCORES)), trace=trace)
    out = assemble_output(br.results)
    if trace:
        return out, br
    return out



# revision 5
# speedup vs baseline: 1.7800x; 1.7800x over previous
"""Multi-head attention (B=2, S=2048, D=1024, H=16) on 8 TRN2 cores.

Sharding: core c -> batch b = c//4, head-group g = c%4 (heads 4g..4g+3,
projection dims 256g..256g+256). Each core computes a partial output
projection over its own 256 head-dims; per-512-token-chunk 4-core
ReduceScatter(add) in bf16 sums the partials and hands each core output
dims 256r..256r+256; collectives overlap later compute.

Key optimizations over the fp32r baseline:
  * all matmul inputs bf16 (fp32r streams ~2 cycles/col on HW; bf16 ~1)
  * host-side key/value compaction: mask==1 tokens contribute exp(-1e9)=0
    exactly, so K/V (and their projections, QK/AV matmuls, and exp work)
    only cover the ~50% surviving tokens, padded to a 128 multiple; pad
    tokens get zero V rows and zero aug columns so they add nothing to
    numerator or denominator
  * s4-half (1024-token) restructure: attention for half 0, then its
    out-projection + ReduceScatter overlap attention for half 1
  * per-head softmax denominators via 4 aug ones-columns on V (pso rows
    64..67), one reciprocal per half, selector-matmul broadcast, DVE
    normalize in-place on at_sb
"""

import numpy as np
from contextlib import ExitStack

import ml_dtypes

import concourse.bass as bass
import concourse.tile as tile
from concourse import mybir
from concourse._compat import with_exitstack

F32 = mybir.dt.float32
BF16 = mybir.dt.bfloat16
AF = mybir.ActivationFunctionType

B, S, D = 2, 2048, 1024
NCORES, GROUP = 8, 4
DG = D // GROUP          # 256 projection dims per core
NH = 4                   # heads per core
DH = 64
SQ = 512                 # sq chunk (PSUM bank width in fp32)
NSQ = S // SQ            # 4
SKT = 128                # sk tile
KT = 128                 # contraction tile
NKT = D // KT            # 8
NAUG = 4                 # aug ones-columns per head (col 64+h hot)
VW = DH + NAUG           # 68 v_aug cols per head
SCALE = 0.125            # 1/sqrt(64)


@with_exitstack
def _mha(ctx: ExitStack, tc: "tile.TileContext", nsk: int, out, xq, xk, xv,
         wq, wk, wv, wo, sel, aug, augt):
    nc = tc.nc
    P = 128
    SKP = nsk * SKT

    # ---- persistent SBUF ----
    persist = ctx.enter_context(tc.tile_pool(name="persist", bufs=1))

    def T(shape, name, dt=BF16):
        return persist.tile(shape, dt, name=name, tag=name)

    wq_sb = T([P, NKT * DG], "wq_sb")
    wk_sb = T([P, NKT * DG], "wk_sb")
    wv_sb = T([P, NKT * DG], "wv_sb")
    wo_sb = T([P, 2 * D], "wo_sb")
    xq_sb = T([P, NKT * S], "xq_sb")
    xk_sb = T([P, NKT * SKP], "xk_sb")
    xv_sb = T([P, NKT * SKP], "xv_sb")
    q_sb = T([P, 2 * S], "q_sb")
    k_sb = T([P, 2 * SKP], "k_sb")
    v_sb = T([P, nsk * NH * VW], "v_sb")
    at_sb = T([P, 2 * S], "at_sb")
    den_sb = T([NAUG, S], "den_sb", F32)
    rec_f = T([NAUG, S], "rec_f", F32)
    rec_r = T([NAUG, S], "rec_r")
    sel_sb = T([NAUG, NH * DH], "sel_sb")
    aug_sb = T([P, NH * NAUG], "aug_sb")
    augt_sb = T([P, NH * NAUG], "augt_sb")
    nc.vector.memset(den_sb[:], 0.0)

    for k in range(NKT):
        nc.sync.dma_start(wq_sb[:, bass.ts(k, DG)], wq[bass.ts(k, P), :])
        nc.sync.dma_start(wk_sb[:, bass.ts(k, DG)], wk[bass.ts(k, P), :])
        nc.sync.dma_start(wv_sb[:, bass.ts(k, DG)], wv[bass.ts(k, P), :])
    for k in range(NKT):
        nc.sync.dma_start(xq_sb[:, bass.ts(k, S)], xq[bass.ts(k, P), :])
    for k in range(NKT):
        nc.sync.dma_start(xk_sb[:, bass.ts(k, SKP)], xk[bass.ts(k, P), :])
    for k in range(NKT):
        nc.sync.dma_start(xv_sb[:, bass.ts(k, SKP)], xv[bass.ts(k, P), :])
    for k in range(2):
        nc.sync.dma_start(wo_sb[:, bass.ts(k, D)], wo[bass.ts(k, P), :])
    nc.sync.dma_start(sel_sb[:], sel[:, :])
    nc.sync.dma_start(aug_sb[:], aug[:, :])
    nc.sync.dma_start(augt_sb[:], augt[:, :])

    # column chunks (<=512) covering the compacted key range
    kchunks = []
    off = 0
    while off < SKP:
        w = min(SQ, SKP - off)
        kchunks.append((off, w))
        off += w

    # ---- phase 1: projections (all bf16) ----
    with tc.tile_pool(name="ppqk", bufs=4, space="PSUM") as ppqk, \
         tc.tile_pool(name="ppv", bufs=2, space="PSUM") as ppv:
        # K projection first: attention needs all of k_sb before head 0.
        for (off, w) in kchunks:
            for d2 in range(2):
                ps = ppqk.tile([P, SQ], F32, name="ps")
                for k in range(NKT):
                    nc.tensor.matmul(
                        ps[:, :w],
                        lhsT=wk_sb[:, bass.ds(k * DG + d2 * P, P)],
                        rhs=xk_sb[:, bass.ds(k * SKP + off, w)],
                        start=(k == 0),
                        stop=(k == NKT - 1),
                    )
                nc.vector.tensor_copy(
                    k_sb[:, bass.ds(d2 * SKP + off, w)], ps[:, :w]
                )
        # Q projection
        for s4 in range(NSQ):
            for d2 in range(2):
                ps = ppqk.tile([P, SQ], F32, name="ps")
                for k in range(NKT):
                    nc.tensor.matmul(
                        ps[:],
                        lhsT=wq_sb[:, bass.ds(k * DG + d2 * P, P)],
                        rhs=xq_sb[:, bass.ds(k * S + s4 * SQ, SQ)],
                        start=(k == 0),
                        stop=(k == NKT - 1),
                    )
                nc.vector.tensor_copy(
                    q_sb[:, bass.ds(d2 * S + s4 * SQ, SQ)], ps[:]
                )
        # V projection, token-major, with aug ones-columns appended per head
        for st in range(nsk):
            psv = ppv.tile([P, DG], F32, name="psv")
            for k in range(NKT):
                nc.tensor.matmul(
                    psv[:],
                    lhsT=xv_sb[:, bass.ds(k * SKP + st * SKT, SKT)],
                    rhs=wv_sb[:, bass.ts(k, DG)],
                    start=(k == 0),
                    stop=(k == NKT - 1),
                )
            base = st * NH * VW
            v3 = v_sb[:, bass.ds(base, NH * VW)].rearrange(
                "p (h w) -> p h w", w=VW)
            nc.vector.tensor_copy(
                v3[:, :, 0:DH], psv[:].rearrange("p (h d) -> p h d", h=NH)
            )
            a_src = augt_sb if st == nsk - 1 else aug_sb
            nc.vector.tensor_copy(
                v3[:, :, DH:VW], a_src[:].rearrange("p (h d) -> p h d", h=NH)
            )

    # ---- phase 2+3: attention halves with overlapped out-proj + RS ----
    dram = ctx.enter_context(tc.tile_pool(name="dram", bufs=1, space="DRAM"))
    rs_in = [dram.tile([D, SQ], BF16, name=f"rs_in{i}", tag=f"rs_in{i}")
             for i in range(NSQ)]
    rs_out = [dram.tile([DG, SQ], BF16, name=f"rs_out{i}", tag=f"rs_out{i}")
              for i in range(NSQ)]

    for s4h in range(2):
        with tc.tile_pool(name="expp", bufs=3) as exp_pool, \
             tc.tile_pool(name="pslp", bufs=2, space="PSUM") as psl_pool, \
             tc.tile_pool(name="psop", bufs=2, space="PSUM") as pso_pool:
            for h in range(NH):
                pr, po = h // 2, (h % 2) * DH
                pso = [pso_pool.tile([VW, SQ], F32, name=f"pso{i}")
                       for i in range(2)]

                def emit_av(item):
                    ex_t, sk_i = item
                    for i in range(2):
                        nc.tensor.matmul(
                            pso[i][:],
                            lhsT=v_sb[:, bass.ds(sk_i * NH * VW + h * VW, VW)],
                            rhs=ex_t[:, bass.ts(i, SQ)],
                            start=(sk_i == 0),
                            stop=(sk_i == nsk - 1),
                            skip_group_check=True,
                        )

                prev = None
                for sk in range(nsk):
                    psl = psl_pool.tile([P, 2 * SQ], F32, name="psl")
                    for i in range(2):
                        s4 = s4h * 2 + i
                        nc.tensor.matmul(
                            psl[:, bass.ts(i, SQ)],
                            lhsT=k_sb[bass.ds(po, DH),
                                      bass.ds(pr * SKP + sk * SKT, SKT)],
                            rhs=q_sb[bass.ds(po, DH),
                                     bass.ds(pr * S + s4 * SQ, SQ)],
                            start=True,
                            stop=True,
                        )
                    ex = exp_pool.tile([P, 2 * SQ], BF16, name="ex")
                    nc.scalar.activation(ex[:], psl[:], AF.Exp, scale=SCALE)
                    if prev is not None:
                        emit_av(prev)
                    prev = (ex, sk)
                emit_av(prev)

                for i in range(2):
                    s4 = s4h * 2 + i
                    nc.vector.tensor_copy(
                        at_sb[bass.ds(po, DH), bass.ds(pr * S + s4 * SQ, SQ)],
                        pso[i][bass.ds(0, DH), :],
                    )
                    nc.vector.tensor_add(
                        den_sb[:, bass.ts(s4, SQ)],
                        den_sb[:, bass.ts(s4, SQ)],
                        pso[i][bass.ds(DH, NAUG), :],
                    )

        # normalize + out-proj + ReduceScatter for this half's two chunks
        nc.vector.reciprocal_approx_fast(
            rec_f[:, bass.ds(s4h * 2 * SQ, 2 * SQ)],
            den_sb[:, bass.ds(s4h * 2 * SQ, 2 * SQ)],
        )
        nc.vector.tensor_copy(
            rec_r[:, bass.ds(s4h * 2 * SQ, 2 * SQ)],
            rec_f[:, bass.ds(s4h * 2 * SQ, 2 * SQ)],
        )

        with tc.tile_pool(name="psb", bufs=4, space="PSUM") as psb_pool, \
             tc.tile_pool(name="fin", bufs=4) as fin_pool, \
             tc.tile_pool(name="psf", bufs=4, space="PSUM") as psf_pool:
            for i in range(2):
                s4 = s4h * 2 + i
                for h in range(NH):
                    pr, po = h // 2, (h % 2) * DH
                    pb = psb_pool.tile([DH, SQ], F32, name="pb")
                    nc.tensor.matmul(
                        pb[:],
                        lhsT=sel_sb[:, bass.ts(h, DH)],
                        rhs=rec_r[:, bass.ds(s4 * SQ, SQ)],
                        start=True,
                        stop=True,
                    )
                    nc.vector.tensor_mul(
                        at_sb[bass.ds(po, DH), bass.ds(pr * S + s4 * SQ, SQ)],
                        at_sb[bass.ds(po, DH), bass.ds(pr * S + s4 * SQ, SQ)],
                        pb[:],
                    )
            for i in range(2):
                s4 = s4h * 2 + i
                for do8 in range(NKT):
                    psf = psf_pool.tile([P, SQ], F32, name="psf")
                    for kt in range(2):
                        nc.tensor.matmul(
                            psf[:],
                            lhsT=wo_sb[:, bass.ds(kt * D + do8 * P, P)],
                            rhs=at_sb[:, bass.ds(kt * S + s4 * SQ, SQ)],
                            start=(kt == 0),
                            stop=(kt == 1),
                        )
                    ot = fin_pool.tile([P, SQ], BF16, name="ot")
                    nc.vector.tensor_copy(ot[:], psf[:])
                    nc.sync.dma_start(rs_in[s4][bass.ts(do8, P), :], ot[:])
                nc.gpsimd.collective_compute(
                    "ReduceScatter",
                    mybir.AluOpType.add,
                    replica_groups=[[0, 1, 2, 3], [4, 5, 6, 7]],
                    ins=[rs_in[s4].opt()],
                    outs=[rs_out[s4].opt()],
                )
                ro = fin_pool.tile([P, 2 * SQ], BF16, name="ro")
                for half in range(2):
                    nc.sync.dma_start(
                        ro[:, bass.ts(half, SQ)], rs_out[s4][bass.ts(half, P), :]
                    )
                of = fin_pool.tile([P, 2 * SQ], F32, name="of")
                nc.vector.tensor_copy(of[:], ro[:])
                for half in range(2):
                    nc.sync.dma_start(
                        out[bass.ts(half, P), bass.ts(s4, SQ)],
                        of[:, bass.ts(half, SQ)],
                    )


def build_program(nsk: int):
    from concourse import bacc

    SKP = nsk * SKT
    nc = bacc.Bacc("TRN2", target_bir_lowering=False, debug=False,
                   num_devices=NCORES)
    aps = {}
    for nm, shp, dt in (
        ("xq", [D, S], BF16),
        ("xk", [D, SKP], BF16),
        ("xv", [D, SKP], BF16),
        ("wq", [D, DG], BF16),
        ("wk", [D, DG], BF16),
        ("wv", [D, DG], BF16),
        ("wo", [DG, D], BF16),
        ("sel", [NAUG, NH * DH], BF16),
        ("aug", [128, NH * NAUG], BF16),
        ("augt", [128, NH * NAUG], BF16),
    ):
        aps[nm] = nc.dram_tensor(nm, shp, dt, kind="ExternalInput").ap()
    out = nc.dram_tensor("out", [DG, S], F32, kind="ExternalOutput").ap()
    with tile.TileContext(nc) as tc:
        _mha(tc, nsk, out, **aps)
    nc.finalize()
    return nc


_NC_CACHE = {}


def _get_program(nsk: int):
    if nsk not in _NC_CACHE:
        _NC_CACHE[nsk] = build_program(nsk)
    return _NC_CACHE[nsk]


def make_in_maps(query, key, value, mask, Wq, Wk, Wv, Wo):
    bf = ml_dtypes.bfloat16
    keep = [np.nonzero(mask[b] == 0)[0] for b in range(B)]
    nsk = max(1, int(np.ceil(max(len(kk) for kk in keep) / SKT)))
    SKP = nsk * SKT

    xT, xkT, xvT, augt = {}, {}, {}, {}
    for b in range(B):
        xT[b] = np.ascontiguousarray(query[b].T.astype(bf))
        nk = len(keep[b])
        kb = np.zeros((SKP, D), dtype=np.float32)
        vb = np.zeros((SKP, D), dtype=np.float32)
        kb[:nk] = key[b][keep[b]]
        vb[:nk] = value[b][keep[b]]
        xkT[b] = np.ascontiguousarray(kb.T.astype(bf))
        xvT[b] = np.ascontiguousarray(vb.T.astype(bf))
        # aug for the last tile: zero rows for pad tokens
        at = np.zeros((128, NH * NAUG), dtype=np.float32)
        valid = nk - (nsk - 1) * SKT
        for h in range(NH):
            at[:valid, h * NAUG + h] = 1.0
        augt[b] = at.astype(bf)

    sel = np.zeros((NAUG, NH * DH), dtype=np.float32)
    aug = np.zeros((128, NH * NAUG), dtype=np.float32)
    for h in range(NH):
        sel[h, h * DH:(h + 1) * DH] = 1.0
        aug[:, h * NAUG + h] = 1.0
    sel = sel.astype(bf)
    aug = aug.astype(bf)

    in_maps = []
    for c in range(NCORES):
        b, g = divmod(c, GROUP)
        in_maps.append(
            {
                "xq": xT[b],
                "xk": xkT[b],
                "xv": xvT[b],
                "wq": np.ascontiguousarray(Wq[g * DG:(g + 1) * DG, :].T.astype(bf)),
                "wk": np.ascontiguousarray(Wk[g * DG:(g + 1) * DG, :].T.astype(bf)),
                "wv": np.ascontiguousarray(Wv[g * DG:(g + 1) * DG, :].T.astype(bf)),
                "wo": np.ascontiguousarray(Wo[:, g * DG:(g + 1) * DG].T.astype(bf)),
                "sel": sel,
                "aug": aug,
                "augt": augt[b],
            }
        )
    return in_maps, nsk


def assemble_output(results):
    out = np.empty((B, S, D), dtype=np.float32)
    for c in range(NCORES):
        b, r = divmod(c, GROUP)
        out[b, :, r * DG:(r + 1) * DG] = results[c]["out"].T
    return out


def kernel(query, key, value, mask, Wq, bq, Wk, bk, Wv, bv, Wo, bo, trace=False):
    from concourse.bass_utils import run_bass_kernel_spmd

    in_maps, nsk = make_in_maps(
        np.asarray(query), np.asarray(key), np.asarray(value), np.asarray(mask),
        np.asarray(Wq), np.asarray(Wk), np.asarray(Wv), np.asarray(Wo),
    )
    nc = _get_program(nsk)
    br = run_bass_kernel_spmd(nc, in_maps, list(range(NCORES)), trace=trace)
    out = assemble_output(br.results)
    if trace:
        return out, br
    return out


# revision 8
# speedup vs baseline: 2.2976x; 1.2908x over previous
"""Multi-head attention (B=2, S=2048, D=1024, H=16) on 8 TRN2 cores.

Sharding: core c -> batch b = c//4, head-group g = c%4 (heads 4g..4g+3,
projection dims 256g..256g+256). Each core computes normalized attention
outputs for its 4 heads; per-512-token-chunk 4-core AllGather (bf16)
shares them, then every core redundantly computes the out-projection for
its own 256 output dims over the full 1024 head-dims (fp32 PSUM
accumulate). Collectives overlap later compute.

Key optimizations over the fp32r baseline:
  * all matmul inputs bf16 (fp32r streams ~2 cycles/col on HW; bf16 ~1)
  * host-side key/value compaction: mask==1 tokens contribute exp(-1e9)=0
    exactly, so K/V (projections, QK/AV matmuls, exp) only cover the ~50%
    surviving tokens, padded to a 128 multiple; pad tokens get zero V
    rows and zero aug columns so they add nothing to numerator or
    denominator, and the exp bias/mask disappears entirely
  * s4-half (1024-token) structure: half 0's normalize + AllGather overlap
    half 1's attention; chunk 0/1 out-projections are issued after half 1's
    attention so the PE never waits on a collective
  * per-head softmax denominators via 4 aug ones-columns on V (pso rows
    64..67), one reciprocal per half, selector-matmul broadcast (packed
    4-heads-per-PSUM-tile), two DVE multiplies per chunk
  * all tile pools hoisted to kernel scope: PSUM = psl(2x2 banks) +
    pso0/1(2x1 bank each) = 8 banks, with sel-broadcast and out-proj
    tiles drawing from the psl tag
"""

import numpy as np
from contextlib import ExitStack

import ml_dtypes

import concourse.bass as bass
import concourse.tile as tile
from concourse import mybir
from concourse._compat import with_exitstack

F32 = mybir.dt.float32
BF16 = mybir.dt.bfloat16
AF = mybir.ActivationFunctionType

B, S, D = 2, 2048, 1024
NCORES, GROUP = 8, 4
DG = D // GROUP          # 256 projection dims per core
NH = 4                   # heads per core
DH = 64
SQ = 512                 # sq chunk (PSUM bank width in fp32)
NSQ = S // SQ            # 4
SKT = 128                # sk tile
KT = 128                 # contraction tile
NKT = D // KT            # 8
NAUG = 4                 # aug ones-columns per head (col 64+h hot)
VW = DH + NAUG           # 68 v_aug cols per head
SCALE = 0.125            # 1/sqrt(64)


@with_exitstack
def _mha(ctx: ExitStack, tc: "tile.TileContext", nsk: int, out, xq, xk, xv,
         wq, wk, wv, wof, sel, aug, augt):
    nc = tc.nc
    P = 128
    SKP = nsk * SKT

    # ---- persistent SBUF ----
    persist = ctx.enter_context(tc.tile_pool(name="persist", bufs=1))

    def T(shape, name, dt=BF16):
        return persist.tile(shape, dt, name=name, tag=name)

    wq_sb = T([P, NKT * DG], "wq_sb")
    wk_sb = T([P, NKT * DG], "wk_sb")
    wv_sb = T([P, NKT * DG], "wv_sb")
    wof_sb = T([P, NKT * DG], "wof_sb")
    xq_sb = T([P, NKT * S], "xq_sb")
    xk_sb = T([P, NKT * SKP], "xk_sb")
    xv_sb = T([P, NKT * SKP], "xv_sb")
    q_sb = T([P, 2 * S], "q_sb")
    k_sb = T([P, 2 * SKP], "k_sb")
    v_sb = T([P, nsk * NH * VW], "v_sb")
    at_sb = T([P, 2 * S], "at_sb")
    den_sb = T([NAUG, S], "den_sb", F32)
    rec_f = T([NAUG, S], "rec_f", F32)
    rec_r = T([NAUG, S], "rec_r")
    sel_sb = T([NAUG, NH * DH], "sel_sb")
    aug_sb = T([P, NH * NAUG], "aug_sb")
    augt_sb = T([P, NH * NAUG], "augt_sb")
    nc.vector.memset(den_sb[:], 0.0)

    # interleave weight/input loads so K-projection can start almost
    # immediately; K first (attention needs all of k_sb), then Q, then V
    for k in range(NKT):
        nc.sync.dma_start(wk_sb[:, bass.ts(k, DG)], wk[bass.ts(k, P), :])
        nc.sync.dma_start(xk_sb[:, bass.ts(k, SKP)], xk[bass.ts(k, P), :])
    for k in range(NKT):
        nc.sync.dma_start(wq_sb[:, bass.ts(k, DG)], wq[bass.ts(k, P), :])
        nc.sync.dma_start(xq_sb[:, bass.ts(k, S)], xq[bass.ts(k, P), :])
    for k in range(NKT):
        nc.sync.dma_start(wv_sb[:, bass.ts(k, DG)], wv[bass.ts(k, P), :])
        nc.sync.dma_start(xv_sb[:, bass.ts(k, SKP)], xv[bass.ts(k, P), :])
    for k in range(NKT):
        nc.sync.dma_start(wof_sb[:, bass.ts(k, DG)], wof[bass.ts(k, P), :])
    nc.sync.dma_start(sel_sb[:], sel[:, :])
    nc.sync.dma_start(aug_sb[:], aug[:, :])
    nc.sync.dma_start(augt_sb[:], augt[:, :])

    # column chunks (<=512) covering the compacted key range
    kchunks = []
    off = 0
    while off < SKP:
        w = min(SQ, SKP - off)
        kchunks.append((off, w))
        off += w

    # ---- hoisted pools (stable buffers across both halves) ----
    exp_pool = ctx.enter_context(tc.tile_pool(name="expp", bufs=3))
    psl_pool = ctx.enter_context(tc.tile_pool(name="pslp", bufs=2, space="PSUM"))
    pso_pool = ctx.enter_context(tc.tile_pool(name="psop", bufs=2, space="PSUM"))
    ag_pool = ctx.enter_context(tc.tile_pool(name="agp", bufs=2))
    fin_pool = ctx.enter_context(tc.tile_pool(name="fin", bufs=2))

    dram = ctx.enter_context(tc.tile_pool(name="dram", bufs=1, space="DRAM"))
    ag_in = [dram.tile([DG, SQ], BF16, name=f"ag_in{i}", tag=f"ag_in{i}")
             for i in range(NSQ)]
    ag_out = [dram.tile([D, SQ], BF16, name=f"ag_out{i}", tag=f"ag_out{i}")
              for i in range(NSQ)]

    def psl_tile():
        return psl_pool.tile([P, 2 * SQ], F32, name="psl", tag="psl")

    # ---- phase 1: projections (all bf16) ----
    # K projection first: attention needs all of k_sb before head 0.
    for (off, w) in kchunks:
        for d2 in range(2):
            ps = psl_tile()
            for k in range(NKT):
                nc.tensor.matmul(
                    ps[:, :w],
                    lhsT=wk_sb[:, bass.ds(k * DG + d2 * P, P)],
                    rhs=xk_sb[:, bass.ds(k * SKP + off, w)],
                    start=(k == 0),
                    stop=(k == NKT - 1),
                )
            nc.vector.tensor_copy(
                k_sb[:, bass.ds(d2 * SKP + off, w)], ps[:, :w]
            )
    # Q projection (two 512-chunks per PSUM tile)
    for s4p in range(2):
        for d2 in range(2):
            ps = psl_tile()
            for i in range(2):
                s4 = s4p * 2 + i
                for k in range(NKT):
                    nc.tensor.matmul(
                        ps[:, bass.ts(i, SQ)],
                        lhsT=wq_sb[:, bass.ds(k * DG + d2 * P, P)],
                        rhs=xq_sb[:, bass.ds(k * S + s4 * SQ, SQ)],
                        start=(k == 0),
                        stop=(k == NKT - 1),
                        skip_group_check=True,
                    )
            nc.vector.tensor_copy(
                q_sb[:, bass.ds(d2 * S + s4p * 2 * SQ, 2 * SQ)], ps[:]
            )
    # V projection, token-major, with aug ones-columns appended per head
    for st in range(nsk):
        pst = psl_tile()
        psv = pst[:, 0:DG]
        for k in range(NKT):
            nc.tensor.matmul(
                psv[:],
                lhsT=xv_sb[:, bass.ds(k * SKP + st * SKT, SKT)],
                rhs=wv_sb[:, bass.ts(k, DG)],
                start=(k == 0),
                stop=(k == NKT - 1),
            )
        base = st * NH * VW
        v3 = v_sb[:, bass.ds(base, NH * VW)].rearrange(
            "p (h w) -> p h w", w=VW)
        nc.vector.tensor_copy(
            v3[:, :, 0:DH], psv[:].rearrange("p (h d) -> p h d", h=NH)
        )
        a_src = augt_sb if st == nsk - 1 else aug_sb
        nc.vector.tensor_copy(
            v3[:, :, DH:VW], a_src[:].rearrange("p (h d) -> p h d", h=NH)
        )

    # ---- out-projection for one 512-token chunk (after its AllGather) ----
    def outproj(s4):
        ag_sb = ag_pool.tile([P, NKT * SQ], BF16, name="ag_sb", tag="ag_sb")
        for k in range(NKT):
            nc.sync.dma_start(
                ag_sb[:, bass.ts(k, SQ)], ag_out[s4][bass.ts(k, P), :]
            )
        pf = psl_tile()
        for od2 in range(2):
            for k in range(NKT):
                nc.tensor.matmul(
                    pf[:, bass.ts(od2, SQ)],
                    lhsT=wof_sb[:, bass.ds(k * DG + od2 * P, P)],
                    rhs=ag_sb[:, bass.ts(k, SQ)],
                    start=(k == 0),
                    stop=(k == NKT - 1),
                    skip_group_check=True,
                )
        of = fin_pool.tile([P, 2 * SQ], F32, name="of", tag="of")
        nc.vector.tensor_copy(of[:], pf[:])
        for od2 in range(2):
            nc.sync.dma_start(
                out[bass.ts(od2, P), bass.ts(s4, SQ)],
                of[:, bass.ts(od2, SQ)],
            )

    # ---- phase 2: attention halves; normalize + AllGather overlap ----
    for s4h in range(2):
        for h in range(NH):
            pr, po = h // 2, (h % 2) * DH
            pso = [pso_pool.tile([VW, SQ], F32, name=f"pso{i}", tag=f"pso{i}")
                   for i in range(2)]

            def emit_av(item):
                ex_t, sk_i = item
                for i in range(2):
                    nc.tensor.matmul(
                        pso[i][:],
                        lhsT=v_sb[:, bass.ds(sk_i * NH * VW + h * VW, VW)],
                        rhs=ex_t[:, bass.ts(i, SQ)],
                        start=(sk_i == 0),
                        stop=(sk_i == nsk - 1),
                        skip_group_check=True,
                    )

            prev = None
            for sk in range(nsk):
                psl = psl_tile()
                for i in range(2):
                    s4 = s4h * 2 + i
                    nc.tensor.matmul(
                        psl[:, bass.ts(i, SQ)],
                        lhsT=k_sb[bass.ds(po, DH),
                                  bass.ds(pr * SKP + sk * SKT, SKT)],
                        rhs=q_sb[bass.ds(po, DH),
                                 bass.ds(pr * S + s4 * SQ, SQ)],
                        start=True,
                        stop=True,
                    )
                ex = exp_pool.tile([P, 2 * SQ], BF16, name="ex")
                nc.scalar.activation(ex[:], psl[:], AF.Exp, scale=SCALE)
                if prev is not None:
                    emit_av(prev)
                prev = (ex, sk)
            emit_av(prev)

            for i in range(2):
                s4 = s4h * 2 + i
                nc.vector.tensor_copy(
                    at_sb[bass.ds(po, DH), bass.ds(pr * S + s4 * SQ, SQ)],
                    pso[i][bass.ds(0, DH), :],
                )
                nc.vector.tensor_add(
                    den_sb[:, bass.ts(s4, SQ)],
                    den_sb[:, bass.ts(s4, SQ)],
                    pso[i][bass.ds(DH, NAUG), :],
                )

        # normalize this half's two chunks, push to DRAM, AllGather
        nc.vector.reciprocal_approx_fast(
            rec_f[:, bass.ds(s4h * 2 * SQ, 2 * SQ)],
            den_sb[:, bass.ds(s4h * 2 * SQ, 2 * SQ)],
        )
        nc.vector.tensor_copy(
            rec_r[:, bass.ds(s4h * 2 * SQ, 2 * SQ)],
            rec_f[:, bass.ds(s4h * 2 * SQ, 2 * SQ)],
        )
        for i in range(2):
            s4 = s4h * 2 + i
            # broadcast each head's reciprocal row to its 64 partitions:
            # head h -> nb[(h%2)*64:, (h//2)*512:], matching at_sb layout
            nb = psl_tile()
            for h in range(NH):
                pr, po = h // 2, (h % 2) * DH
                nc.tensor.matmul(
                    nb[bass.ds(po, DH), bass.ts(pr, SQ)],
                    lhsT=sel_sb[:, bass.ts(h, DH)],
                    rhs=rec_r[:, bass.ds(s4 * SQ, SQ)],
                    start=True,
                    stop=True,
                    skip_group_check=True,
                )
            for pr in range(2):
                nc.vector.tensor_mul(
                    at_sb[:, bass.ds(pr * S + s4 * SQ, SQ)],
                    at_sb[:, bass.ds(pr * S + s4 * SQ, SQ)],
                    nb[:, bass.ts(pr, SQ)],
                )
            for h in range(NH):
                pr, po = h // 2, (h % 2) * DH
                nc.sync.dma_start(
                    ag_in[s4][bass.ts(h, DH), :],
                    at_sb[bass.ds(po, DH), bass.ds(pr * S + s4 * SQ, SQ)],
                )
            nc.gpsimd.collective_compute(
                "AllGather",
                mybir.AluOpType.bypass,
                replica_groups=[[0, 1, 2, 3], [4, 5, 6, 7]],
                ins=[ag_in[s4].opt()],
                outs=[ag_out[s4].opt()],
            )
        if s4h == 1:
            # chunk 0/1 data arrived during half-1 attention; 2/3 drain now
            for s4 in range(NSQ):
                outproj(s4)


def build_program(nsk: int):
    from concourse import bacc

    SKP = nsk * SKT
    nc = bacc.Bacc("TRN2", target_bir_lowering=False, debug=False,
                   num_devices=NCORES)
    aps = {}
    for nm, shp, dt in (
        ("xq", [D, S], BF16),
        ("xk", [D, SKP], BF16),
        ("xv", [D, SKP], BF16),
        ("wq", [D, DG], BF16),
        ("wk", [D, DG], BF16),
        ("wv", [D, DG], BF16),
        ("wof", [D, DG], BF16),
        ("sel", [NAUG, NH * DH], BF16),
        ("aug", [128, NH * NAUG], BF16),
        ("augt", [128, NH * NAUG], BF16),
    ):
        aps[nm] = nc.dram_tensor(nm, shp, dt, kind="ExternalInput").ap()
    out = nc.dram_tensor("out", [DG, S], F32, kind="ExternalOutput").ap()
    with tile.TileContext(nc) as tc:
        _mha(tc, nsk, out, **aps)
    nc.finalize()
    return nc


_NC_CACHE = {}


def _get_program(nsk: int):
    if nsk not in _NC_CACHE:
        _NC_CACHE[nsk] = build_program(nsk)
    return _NC_CACHE[nsk]


def make_in_maps(query, key, value, mask, Wq, Wk, Wv, Wo):
    bf = ml_dtypes.bfloat16
    keep = [np.nonzero(mask[b] == 0)[0] for b in range(B)]
    nsk = max(1, int(np.ceil(max(len(kk) for kk in keep) / SKT)))
    SKP = nsk * SKT

    xT, xkT, xvT, augt = {}, {}, {}, {}
    for b in range(B):
        xT[b] = np.ascontiguousarray(query[b].T.astype(bf))
        nk = len(keep[b])
        kb = np.zeros((SKP, D), dtype=np.float32)
        vb = np.zeros((SKP, D), dtype=np.float32)
        kb[:nk] = key[b][keep[b]]
        vb[:nk] = value[b][keep[b]]
        xkT[b] = np.ascontiguousarray(kb.T.astype(bf))
        xvT[b] = np.ascontiguousarray(vb.T.astype(bf))
        # aug for the last tile: zero rows for pad tokens
        at = np.zeros((128, NH * NAUG), dtype=np.float32)
        valid = nk - (nsk - 1) * SKT
        for h in range(NH):
            at[:valid, h * NAUG + h] = 1.0
        augt[b] = at.astype(bf)

    sel = np.zeros((NAUG, NH * DH), dtype=np.float32)
    aug = np.zeros((128, NH * NAUG), dtype=np.float32)
    for h in range(NH):
        sel[h, h * DH:(h + 1) * DH] = 1.0
        aug[:, h * NAUG + h] = 1.0
    sel = sel.astype(bf)
    aug = aug.astype(bf)

    in_maps = []
    for c in range(NCORES):
        b, g = divmod(c, GROUP)
        in_maps.append(
            {
                "xq": xT[b],
                "xk": xkT[b],
                "xv": xvT[b],
                "wq": np.ascontiguousarray(Wq[g * DG:(g + 1) * DG, :].T.astype(bf)),
                "wk": np.ascontiguousarray(Wk[g * DG:(g + 1) * DG, :].T.astype(bf)),
                "wv": np.ascontiguousarray(Wv[g * DG:(g + 1) * DG, :].T.astype(bf)),
                "wof": np.ascontiguousarray(Wo[g * DG:(g + 1) * DG, :].T.astype(bf)),
                "sel": sel,
                "aug": aug,
                "augt": augt[b],
            }
        )
    return in_maps, nsk


def assemble_output(results):
    out = np.empty((B, S, D), dtype=np.float32)
    for c in range(NCORES):
        b, r = divmod(c, GROUP)
        out[b, :, r * DG:(r + 1) * DG] = results[c]["out"].T
    return out


def kernel(query, key, value, mask, Wq, bq, Wk, bk, Wv, bv, Wo, bo, trace=False):
    from concourse.bass_utils import run_bass_kernel_spmd

    in_maps, nsk = make_in_maps(
        np.asarray(query), np.asarray(key), np.asarray(value), np.asarray(mask),
        np.asarray(Wq), np.asarray(Wk), np.asarray(Wv), np.asarray(Wo),
    )
    nc = _get_program(nsk)
    br = run_bass_kernel_spmd(nc, in_maps, list(range(NCORES)), trace=trace)
    out = assemble_output(br.results)
    if trace:
        return out, br
    return out


# revision 9
# speedup vs baseline: 2.5036x; 1.0897x over previous
"""Multi-head attention (B=2, S=2048, D=1024, H=16) on 8 TRN2 cores.

Sharding: core c -> batch b = c//4, head-group g = c%4 (heads 4g..4g+3,
projection dims 256g..256g+256). Each core computes normalized attention
outputs for its 4 heads, then a partial out-projection over its own 256
head-dims; per-512-token-chunk 4-core ReduceScatter(add) in bf16 sums
the partials and hands each core output dims 256r..256r+256. Collectives
run on the CC while compute continues; all gather/cast/store work is
deferred to the kernel tail.

Key optimizations over the fp32r baseline:
  * all matmul inputs bf16 (fp32r streams ~2 cycles/col on HW; bf16 ~1)
  * host-side key/value compaction: mask==1 tokens contribute exp(-1e9)=0
    exactly, so K/V (projections, QK/AV matmuls, exp) only cover the ~50%
    surviving tokens, padded to a 128 multiple; pad tokens get zero V
    rows and zero aug columns so they add nothing to numerator or
    denominator, and the exp bias/mask disappears entirely
  * s4-half (1024-token) structure: half 0's normalize + AllGather overlap
    half 1's attention; chunk 0/1 out-projections are issued after half 1's
    attention so the PE never waits on a collective
  * per-head softmax denominators via 4 aug ones-columns on V (pso rows
    64..67), one reciprocal per half, selector-matmul broadcast (packed
    4-heads-per-PSUM-tile), two DVE multiplies per chunk
  * all tile pools hoisted to kernel scope: PSUM = psl(2x2 banks) +
    pso0/1(2x1 bank each) = 8 banks, with sel-broadcast and out-proj
    tiles drawing from the psl tag
"""

import numpy as np
from contextlib import ExitStack

import ml_dtypes

import concourse.bass as bass
import concourse.tile as tile
from concourse import mybir
from concourse._compat import with_exitstack

F32 = mybir.dt.float32
BF16 = mybir.dt.bfloat16
AF = mybir.ActivationFunctionType

B, S, D = 2, 2048, 1024
NCORES, GROUP = 8, 4
DG = D // GROUP          # 256 projection dims per core
NH = 4                   # heads per core
DH = 64
SQ = 512                 # sq chunk (PSUM bank width in fp32)
NSQ = S // SQ            # 4
SKT = 128                # sk tile
KT = 128                 # contraction tile
NKT = D // KT            # 8
NAUG = 4                 # aug ones-columns per head (col 64+h hot)
VW = DH + NAUG           # 68 v_aug cols per head
SCALE = 0.125            # 1/sqrt(64)


@with_exitstack
def _mha(ctx: ExitStack, tc: "tile.TileContext", nsk: int, out, xq, xk, xv,
         wq, wk, wv, wo, sel, aug, augt):
    nc = tc.nc
    P = 128
    SKP = nsk * SKT

    # ---- persistent SBUF ----
    persist = ctx.enter_context(tc.tile_pool(name="persist", bufs=1))

    def T(shape, name, dt=BF16):
        return persist.tile(shape, dt, name=name, tag=name)

    wq_sb = T([P, NKT * DG], "wq_sb")
    wk_sb = T([P, NKT * DG], "wk_sb")
    wv_sb = T([P, NKT * DG], "wv_sb")
    wo_sb = T([P, 2 * D], "wo_sb")
    xq_sb = T([P, NKT * S], "xq_sb")
    xk_sb = T([P, NKT * SKP], "xk_sb")
    xv_sb = T([P, NKT * SKP], "xv_sb")
    q_sb = T([P, 2 * S], "q_sb")
    k_sb = T([P, 2 * SKP], "k_sb")
    v_sb = T([P, nsk * NH * VW], "v_sb")
    at_sb = T([P, 2 * S], "at_sb")
    den_sb = T([NAUG, S], "den_sb", F32)
    rec_f = T([NAUG, S], "rec_f", F32)
    rec_r = T([NAUG, S], "rec_r")
    sel_sb = T([NAUG, NH * DH], "sel_sb")
    aug_sb = T([P, NH * NAUG], "aug_sb")
    augt_sb = T([P, NH * NAUG], "augt_sb")
    nc.vector.memset(den_sb[:], 0.0)

    # interleave weight/input loads so K-projection can start almost
    # immediately; K first (attention needs all of k_sb), then Q, then V
    for k in range(NKT):
        nc.sync.dma_start(wk_sb[:, bass.ts(k, DG)], wk[bass.ts(k, P), :])
        nc.sync.dma_start(xk_sb[:, bass.ts(k, SKP)], xk[bass.ts(k, P), :])
    for k in range(NKT):
        nc.sync.dma_start(wq_sb[:, bass.ts(k, DG)], wq[bass.ts(k, P), :])
        nc.sync.dma_start(xq_sb[:, bass.ts(k, S)], xq[bass.ts(k, P), :])
    for k in range(NKT):
        nc.sync.dma_start(wv_sb[:, bass.ts(k, DG)], wv[bass.ts(k, P), :])
        nc.sync.dma_start(xv_sb[:, bass.ts(k, SKP)], xv[bass.ts(k, P), :])
    for k in range(2):
        nc.sync.dma_start(wo_sb[:, bass.ts(k, D)], wo[bass.ts(k, P), :])
    nc.sync.dma_start(sel_sb[:], sel[:, :])
    nc.sync.dma_start(aug_sb[:], aug[:, :])
    nc.sync.dma_start(augt_sb[:], augt[:, :])

    # column chunks (<=512) covering the compacted key range
    kchunks = []
    off = 0
    while off < SKP:
        w = min(SQ, SKP - off)
        kchunks.append((off, w))
        off += w

    # ---- hoisted pools (stable buffers across both halves) ----
    exp_pool = ctx.enter_context(tc.tile_pool(name="expp", bufs=3))
    psl_pool = ctx.enter_context(tc.tile_pool(name="pslp", bufs=2, space="PSUM"))
    pso_pool = ctx.enter_context(tc.tile_pool(name="psop", bufs=2, space="PSUM"))
    fin_pool = ctx.enter_context(tc.tile_pool(name="fin", bufs=4))

    dram = ctx.enter_context(tc.tile_pool(name="dram", bufs=1, space="DRAM"))
    rs_in = [dram.tile([D, SQ], BF16, name=f"rs_in{i}", tag=f"rs_in{i}")
             for i in range(NSQ)]
    rs_out = [dram.tile([DG, SQ], BF16, name=f"rs_out{i}", tag=f"rs_out{i}")
              for i in range(NSQ)]

    def psl_tile():
        return psl_pool.tile([P, 2 * SQ], F32, name="psl", tag="psl")

    # ---- phase 1: projections (all bf16) ----
    # K projection first: attention needs all of k_sb before head 0.
    for (off, w) in kchunks:
        for d2 in range(2):
            ps = psl_tile()
            for k in range(NKT):
                nc.tensor.matmul(
                    ps[:, :w],
                    lhsT=wk_sb[:, bass.ds(k * DG + d2 * P, P)],
                    rhs=xk_sb[:, bass.ds(k * SKP + off, w)],
                    start=(k == 0),
                    stop=(k == NKT - 1),
                )
            nc.vector.tensor_copy(
                k_sb[:, bass.ds(d2 * SKP + off, w)], ps[:, :w]
            )
    # Q projection (two 512-chunks per PSUM tile)
    for s4p in range(2):
        for d2 in range(2):
            ps = psl_tile()
            for i in range(2):
                s4 = s4p * 2 + i
                for k in range(NKT):
                    nc.tensor.matmul(
                        ps[:, bass.ts(i, SQ)],
                        lhsT=wq_sb[:, bass.ds(k * DG + d2 * P, P)],
                        rhs=xq_sb[:, bass.ds(k * S + s4 * SQ, SQ)],
                        start=(k == 0),
                        stop=(k == NKT - 1),
                        skip_group_check=True,
                    )
            nc.vector.tensor_copy(
                q_sb[:, bass.ds(d2 * S + s4p * 2 * SQ, 2 * SQ)], ps[:]
            )
    # V projection, token-major, with aug ones-columns appended per head
    for st in range(nsk):
        pst = psl_tile()
        psv = pst[:, 0:DG]
        for k in range(NKT):
            nc.tensor.matmul(
                psv[:],
                lhsT=xv_sb[:, bass.ds(k * SKP + st * SKT, SKT)],
                rhs=wv_sb[:, bass.ts(k, DG)],
                start=(k == 0),
                stop=(k == NKT - 1),
            )
        base = st * NH * VW
        v3 = v_sb[:, bass.ds(base, NH * VW)].rearrange(
            "p (h w) -> p h w", w=VW)
        nc.vector.tensor_copy(
            v3[:, :, 0:DH], psv[:].rearrange("p (h d) -> p h d", h=NH)
        )
        a_src = augt_sb if st == nsk - 1 else aug_sb
        nc.vector.tensor_copy(
            v3[:, :, DH:VW], a_src[:].rearrange("p (h d) -> p h d", h=NH)
        )

    # ---- partial out-projection + ReduceScatter for one 512-token chunk ----
    def outproj_rs(s4):
        for dp in range(NKT // 2):
            pf = psl_tile()
            for j in range(2):
                do8 = dp * 2 + j
                for kt in range(2):
                    nc.tensor.matmul(
                        pf[:, bass.ts(j, SQ)],
                        lhsT=wo_sb[:, bass.ds(kt * D + do8 * P, P)],
                        rhs=at_sb[:, bass.ds(kt * S + s4 * SQ, SQ)],
                        start=(kt == 0),
                        stop=(kt == 1),
                        skip_group_check=True,
                    )
            ot = fin_pool.tile([P, 2 * SQ], BF16, name="ot", tag="ot")
            nc.vector.tensor_copy(ot[:], pf[:])
            for j in range(2):
                nc.sync.dma_start(
                    rs_in[s4][bass.ts(dp * 2 + j, P), :], ot[:, bass.ts(j, SQ)]
                )
        nc.gpsimd.collective_compute(
            "ReduceScatter",
            mybir.AluOpType.add,
            replica_groups=[[0, 1, 2, 3], [4, 5, 6, 7]],
            ins=[rs_in[s4].opt()],
            outs=[rs_out[s4].opt()],
        )

    # ---- phase 2: attention halves; normalize + AllGather overlap ----
    for s4h in range(2):
        for h in range(NH):
            pr, po = h // 2, (h % 2) * DH
            pso = [pso_pool.tile([VW, SQ], F32, name=f"pso{i}", tag=f"pso{i}")
                   for i in range(2)]

            def emit_av(item):
                ex_t, sk_i = item
                for i in range(2):
                    nc.tensor.matmul(
                        pso[i][:],
                        lhsT=v_sb[:, bass.ds(sk_i * NH * VW + h * VW, VW)],
                        rhs=ex_t[:, bass.ts(i, SQ)],
                        start=(sk_i == 0),
                        stop=(sk_i == nsk - 1),
                        skip_group_check=True,
                    )

            prev = None
            for sk in range(nsk):
                psl = psl_tile()
                for i in range(2):
                    s4 = s4h * 2 + i
                    nc.tensor.matmul(
                        psl[:, bass.ts(i, SQ)],
                        lhsT=k_sb[bass.ds(po, DH),
                                  bass.ds(pr * SKP + sk * SKT, SKT)],
                        rhs=q_sb[bass.ds(po, DH),
                                 bass.ds(pr * S + s4 * SQ, SQ)],
                        start=True,
                        stop=True,
                    )
                ex = exp_pool.tile([P, 2 * SQ], BF16, name="ex")
                nc.scalar.activation(ex[:], psl[:], AF.Exp, scale=SCALE)
                if prev is not None:
                    emit_av(prev)
                prev = (ex, sk)
            emit_av(prev)

            for i in range(2):
                s4 = s4h * 2 + i
                nc.vector.tensor_copy(
                    at_sb[bass.ds(po, DH), bass.ds(pr * S + s4 * SQ, SQ)],
                    pso[i][bass.ds(0, DH), :],
                )
                nc.vector.tensor_add(
                    den_sb[:, bass.ts(s4, SQ)],
                    den_sb[:, bass.ts(s4, SQ)],
                    pso[i][bass.ds(DH, NAUG), :],
                )

        # normalize this half's two chunks, push to DRAM, AllGather
        nc.vector.reciprocal_approx_fast(
            rec_f[:, bass.ds(s4h * 2 * SQ, 2 * SQ)],
            den_sb[:, bass.ds(s4h * 2 * SQ, 2 * SQ)],
        )
        nc.vector.tensor_copy(
            rec_r[:, bass.ds(s4h * 2 * SQ, 2 * SQ)],
            rec_f[:, bass.ds(s4h * 2 * SQ, 2 * SQ)],
        )
        for i in range(2):
            s4 = s4h * 2 + i
            # broadcast each head's reciprocal row to its 64 partitions:
            # head h -> nb[(h%2)*64:, (h//2)*512:], matching at_sb layout
            nb = psl_tile()
            for h in range(NH):
                pr, po = h // 2, (h % 2) * DH
                nc.tensor.matmul(
                    nb[bass.ds(po, DH), bass.ts(pr, SQ)],
                    lhsT=sel_sb[:, bass.ts(h, DH)],
                    rhs=rec_r[:, bass.ds(s4 * SQ, SQ)],
                    start=True,
                    stop=True,
                    skip_group_check=True,
                )
            for pr in range(2):
                nc.vector.tensor_mul(
                    at_sb[:, bass.ds(pr * S + s4 * SQ, SQ)],
                    at_sb[:, bass.ds(pr * S + s4 * SQ, SQ)],
                    nb[:, bass.ts(pr, SQ)],
                )
            outproj_rs(s4)

    # ---- tail: gather, cast to fp32, store all four chunks ----
    for s4 in range(NSQ):
        ro = fin_pool.tile([P, 2 * SQ], BF16, name="ro", tag="ro")
        for half in range(2):
            nc.sync.dma_start(
                ro[:, bass.ts(half, SQ)], rs_out[s4][bass.ts(half, P), :]
            )
        of = fin_pool.tile([P, 2 * SQ], F32, name="of", tag="of")
        nc.vector.tensor_copy(of[:], ro[:])
        for half in range(2):
            nc.sync.dma_start(
                out[bass.ts(half, P), bass.ts(s4, SQ)],
                of[:, bass.ts(half, SQ)],
            )


def build_program(nsk: int):
    from concourse import bacc

    SKP = nsk * SKT
    nc = bacc.Bacc("TRN2", target_bir_lowering=False, debug=False,
                   num_devices=NCORES)
    aps = {}
    for nm, shp, dt in (
        ("xq", [D, S], BF16),
        ("xk", [D, SKP], BF16),
        ("xv", [D, SKP], BF16),
        ("wq", [D, DG], BF16),
        ("wk", [D, DG], BF16),
        ("wv", [D, DG], BF16),
        ("wo", [DG, D], BF16),
        ("sel", [NAUG, NH * DH], BF16),
        ("aug", [128, NH * NAUG], BF16),
        ("augt", [128, NH * NAUG], BF16),
    ):
        aps[nm] = nc.dram_tensor(nm, shp, dt, kind="ExternalInput").ap()
    out = nc.dram_tensor("out", [DG, S], F32, kind="ExternalOutput").ap()
    with tile.TileContext(nc) as tc:
        _mha(tc, nsk, out, **aps)
    nc.finalize()
    return nc


_NC_CACHE = {}


def _get_program(nsk: int):
    if nsk not in _NC_CACHE:
        _NC_CACHE[nsk] = build_program(nsk)
    return _NC_CACHE[nsk]


def make_in_maps(query, key, value, mask, Wq, Wk, Wv, Wo):
    bf = ml_dtypes.bfloat16
    keep = [np.nonzero(mask[b] == 0)[0] for b in range(B)]
    nsk = max(1, int(np.ceil(max(len(kk) for kk in keep) / SKT)))
    SKP = nsk * SKT

    xT, xkT, xvT, augt = {}, {}, {}, {}
    for b in range(B):
        xT[b] = np.ascontiguousarray(query[b].T.astype(bf))
        nk = len(keep[b])
        kb = np.zeros((SKP, D), dtype=np.float32)
        vb = np.zeros((SKP, D), dtype=np.float32)
        kb[:nk] = key[b][keep[b]]
        vb[:nk] = value[b][keep[b]]
        xkT[b] = np.ascontiguousarray(kb.T.astype(bf))
        xvT[b] = np.ascontiguousarray(vb.T.astype(bf))
        # aug for the last tile: zero rows for pad tokens
        at = np.zeros((128, NH * NAUG), dtype=np.float32)
        valid = nk - (nsk - 1) * SKT
        for h in range(NH):
            at[:valid, h * NAUG + h] = 1.0
        augt[b] = at.astype(bf)

    sel = np.zeros((NAUG, NH * DH), dtype=np.float32)
    aug = np.zeros((128, NH * NAUG), dtype=np.float32)
    for h in range(NH):
        sel[h, h * DH:(h + 1) * DH] = 1.0
        aug[:, h * NAUG + h] = 1.0
    sel = sel.astype(bf)
    aug = aug.astype(bf)

    in_maps = []
    for c in range(NCORES):
        b, g = divmod(c, GROUP)
        in_maps.append(
            {
                "xq": xT[b],
                "xk": xkT[b],
                "xv": xvT[b],
                "wq": np.ascontiguousarray(Wq[g * DG:(g + 1) * DG, :].T.astype(bf)),
                "wk": np.ascontiguousarray(Wk[g * DG:(g + 1) * DG, :].T.astype(bf)),
                "wv": np.ascontiguousarray(Wv[g * DG:(g + 1) * DG, :].T.astype(bf)),
                "wo": np.ascontiguousarray(Wo[:, g * DG:(g + 1) * DG].T.astype(bf)),
                "sel": sel,
                "aug": aug,
                "augt": augt[b],
            }
        )
    return in_maps, nsk


def assemble_output(results):
    out = np.empty((B, S, D), dtype=np.float32)
    for c in range(NCORES):
        b, r = divmod(c, GROUP)
        out[b, :, r * DG:(r + 1) * DG] = results[c]["out"].T
    return out


def kernel(query, key, value, mask, Wq, bq, Wk, bk, Wv, bv, Wo, bo, trace=False):
    from concourse.bass_utils import run_bass_kernel_spmd

    in_maps, nsk = make_in_maps(
        np.asarray(query), np.asarray(key), np.asarray(value), np.asarray(mask),
        np.asarray(Wq), np.asarray(Wk), np.asarray(Wv), np.asarray(Wo),
    )
    nc = _get_program(nsk)
    br = run_bass_kernel_spmd(nc, in_maps, list(range(NCORES)), trace=trace)
    out = assemble_output(br.results)
    if trace:
        return out, br
    return out
